# revision 27
# baseline (speedup 1.0000x reference)
"""Correlation kernel for Trainium2 (raw bass, manual semaphores).

corr[b,i,j,k,l] = sum_c A[b,i,j,c] * B[b,k,l,c]

Batched GEMM per batch element: C_b = A_b @ B_b^T with A_b, B_b of
shape (HW, C) = (2304, 256), C_b of (2304, 2304). Batch (8) is
data-parallel across the 8 NeuronCores, one batch element per core.

Per-core pipeline (engines run concurrently, chasing semaphores):
  SP   : load A, B (one DMA each) -> natural [128pix, 18, 256c];
         store finished [128, 2304] output row-blocks.
  Pool : build the 128x128 identity for PE-transposes.
  PE   : transpose A/B 128x128 blocks into PSUM (identity matmul),
         then the GEMM: per M-tile, 5 output column banks
         (4x512 + 1x256) x 2 K-subtiles of f32r matmuls accumulating
         in PSUM. f32r runs 1 cycle/row (vs 4 for fp32).
  DVE  : copy transpose results PSUM -> SBUF (rounding into f32r
         operand tensors), and finished matmul banks PSUM -> SBUF
         staging for the stores.

The target walrus only supports ONE inline semaphore wait per
engine instruction, so every instruction is arranged to need at most
one (PE waits fuse because all its producers tick the single DVE sem;
extra conditions become standalone EventSemaphore waits).
"""

from contextlib import ExitStack

import numpy as np

import concourse.bass as bass
import concourse.mybir as mybir
from concourse.bass import ts
from concourse.bass_utils import run_bass_kernel_spmd

# Problem shape (hardcoded; the harness always calls with these shapes).
B, H, W, C = 8, 48, 48, 256
HW = H * W  # 2304
P = 128
KT = C // P  # 2 k-subtiles
M_TILES = HW // P  # 18
# Output column banks: 4x512 + 1x256 = 2304; each one PSUM bank (fp32),
# each >=256 wide so f32r matmuls run at full rate.
N_SIZES = [512, 512, 512, 512, 256]
N_OFFS = [0, 512, 1024, 1536, 2048]
N_BANKS = len(N_SIZES)
N_TP = 3  # transpose psum double-buffers (banks): 5 + 3 = 8 banks
N_OUT = 4  # SBUF output staging row-blocks

F32 = mybir.dt.float32
F32R = mybir.dt.float32r

USE_F32R = True
MM_DT = F32R if USE_F32R else F32


def build_bass() -> bass.Bass:
    nc = bass.Bass(trn_type="TRN2")

    a_d = nc.dram_tensor("a", [HW, C], F32, kind="ExternalInput")
    b_d = nc.dram_tensor("b", [HW, C], F32, kind="ExternalInput")
    c_d = nc.dram_tensor("c", [HW, HW], F32, kind="ExternalOutput")

    ident = nc.alloc_sbuf_tensor("ident", [P, P], F32)
    anat = nc.alloc_sbuf_tensor("anat", [P, M_TILES, C], F32)
    bnat = nc.alloc_sbuf_tensor("bnat", [P, M_TILES, C], F32)
    at_all = nc.alloc_sbuf_tensor("at_all", [P, KT, HW], MM_DT)
    bt_all = nc.alloc_sbuf_tensor("bt_all", [P, KT, HW], MM_DT)
    outb = nc.alloc_sbuf_tensor("outb", [P, N_OUT, HW], F32)

    mm_ps = [
        nc.alloc_psum_tensor(f"mmps{j}", [P, N_SIZES[j]], F32) for j in range(N_BANKS)
    ]
    tp_ps = [nc.alloc_psum_tensor(f"tpps{i}", [P, P], F32) for i in range(N_TP)]

    # ---- Static schedule bookkeeping -------------------------------------
    # PE event list, in PE program order. Items:
    #   ("tpose", src, pt, k, dst_ap_fn)  transpose of 128x128 block
    #   ("mm", mt, k, j)                  matmul into bank j
    # DVE copies chase PE 1:1 for transposes; out-copies chase k=1 matmuls.
    pe_events = []
    # A m-tile 0 first, then B per bank (so mt=0 matmuls can start early),
    # then per remaining m-tile its A transposes + matmuls.
    pe_events.append(("tpose", "a", 0, 0))
    pe_events.append(("tpose", "a", 0, 1))
    for j in range(N_BANKS):
        for pt in range(N_OFFS[j] // P, (N_OFFS[j] + N_SIZES[j]) // P):
            for k in range(KT):
                pe_events.append(("tpose", "b", pt, k))
        for k in range(KT):
            pe_events.append(("mm", 0, k, j))
    for mt in range(1, M_TILES):
        for k in range(KT):
            pe_events.append(("tpose", "a", mt, k))
        for k in range(KT):
            for j in range(N_BANKS):
                pe_events.append(("mm", mt, k, j))

    # Assign PE sem ticks: transposes and k=1 matmuls increment s_pe.
    pe_tick = {}
    t = 0
    for ev in pe_events:
        if ev[0] == "tpose" or ev[2] == KT - 1:
            t += 1
            pe_tick[ev] = t
    tpose_list = []  # transpose index -> (src, pt, k)
    tpose_order = {}  # (src, pt, k) -> transpose index (0-based)
    for ev in pe_events:
        if ev[0] == "tpose":
            tpose_order[(ev[1], ev[2], ev[3])] = len(tpose_list)
            tpose_list.append((ev[1], ev[2], ev[3]))

    # DVE stream: copy of transpose i right after PE does it; out-copy
    # (mt, j) after mm(mt, KT-1, j). Emit in the same global order as PE.
    dve_events = []
    for ev in pe_events:
        if ev[0] == "tpose":
            dve_events.append(("tc", ev[1], ev[2], ev[3]))
        elif ev[2] == KT - 1:
            dve_events.append(("oc", ev[1], ev[3]))
    dve_tick = {ev: i + 1 for i, ev in enumerate(dve_events)}

    def tc_tick(src, pt, k):
        return dve_tick[("tc", src, pt, k)]

    def oc_tick(mt, j):
        return dve_tick[("oc", mt, j)]

    def bank_ready_tick(j, k):
        # all B transpose copies for bank j, subtile k
        return max(
            tc_tick("b", pt, k)
            for pt in range(N_OFFS[j] // P, (N_OFFS[j] + N_SIZES[j]) // P)
        )

    def at_dst(pt, k):
        return at_all.ap()[:, k, ts(pt, P)]

    def bt_dst(pt, k):
        return bt_all.ap()[:, k, ts(pt, P)]

    with (
        nc.Block() as block,
        nc.semaphore("s_in") as s_in,
        nc.semaphore("s_pool") as s_pool,
        nc.semaphore("s_pe") as s_pe,
        nc.semaphore("s_dve") as s_dve,
        ExitStack() as stack,
    ):
        s_st = [stack.enter_context(nc.semaphore(f"s_st{i}")) for i in range(N_OUT)]

        @block.sync
        def _(sync: bass.BassEngine):
            sync.dma_start(
                out=anat.ap(), in_=a_d.ap().rearrange("(po pi) c -> pi po c", pi=P)
            ).then_inc(s_in, 16)
            sync.dma_start(
                out=bnat.ap(), in_=b_d.ap().rearrange("(po pi) c -> pi po c", pi=P)
            ).then_inc(s_in, 16)
            for mt in range(M_TILES):
                sync.wait_ge(s_dve, oc_tick(mt, N_BANKS - 1))
                sync.dma_start(
                    out=c_d.ap()[ts(mt, P), :], in_=outb.ap()[:, mt % N_OUT, :]
                ).then_inc(s_st[mt % N_OUT], 16)

        @block.gpsimd
        def _(gpsimd: bass.BassEngine):
            gpsimd.memset(ident.ap(), 0.0).then_inc(s_pool, 1)
            gpsimd.affine_select(
                out=ident.ap(),
                in_=ident.ap(),
                compare_op=mybir.AluOpType.not_equal,
                fill=1.0,
                base=0,
                # out[x, y] = (x - y) != 0 ? 0.0 : 1.0
                pattern=[[-1, P]],
                channel_multiplier=1,
            ).wait_op(s_pool, 1, "sem-ge").then_inc(s_pool, 1)

        @block.tensor
        def _(pe: bass.BassEngine):
            pe.wait_ge(s_pool, 2)
            pe.wait_ge(s_in, 32)
            last_wait = 0
            for ev in pe_events:
                if ev[0] == "tpose":
                    _, src, pt, k = ev
                    i = tpose_order[(src, pt, k)]
                    nat = anat if src == "a" else bnat
                    # recycled transpose psum bank: wait for its drain copy
                    w = 0
                    if i >= N_TP:
                        w = tc_tick(*tpose_list[i - N_TP])
                    ins = pe.transpose(
                        tp_ps[i % N_TP].ap(),
                        nat.ap()[:, pt, ts(k, P)],
                        ident.ap(),
                    )
                    if w > last_wait:
                        ins.wait_op(s_dve, w, "sem-ge")
                        last_wait = w
                    ins.then_inc(s_pe, 1)
                else:
                    _, mt, k, j = ev
                    need = max(
                        tc_tick("a", mt, 0),
                        tc_tick("a", mt, 1),
                        bank_ready_tick(j, k),
                        oc_tick(mt - 1, j) if mt > 0 else 0,
                    )
                    ins = pe.matmul(
                        mm_ps[j].ap(),
                        at_all.ap()[:, k, ts(mt, P)],
                        bt_all.ap()[:, k, N_OFFS[j] : N_OFFS[j] + N_SIZES[j]],
                        start=(k == 0),
                        stop=(k == KT - 1),
                    )
                    if need > last_wait:
                        ins.wait_op(s_dve, need, "sem-ge")
                        last_wait = need
                    if k == KT - 1:
                        ins.then_inc(s_pe, 1)

        @block.vector
        def _(dve: bass.BassEngine):
            last_wait = 0
            for ev in dve_events:
                if ev[0] == "tc":
                    _, src, pt, k = ev
                    i = tpose_order[(src, pt, k)]
                    dst = at_dst(pt, k) if src == "a" else bt_dst(pt, k)
                    w = pe_tick[("tpose", src, pt, k)]
                    ins = dve.tensor_copy(dst, tp_ps[i % N_TP].ap())
                    if w > last_wait:
                        ins.wait_op(s_pe, w, "sem-ge")
                        last_wait = w
                    ins.then_inc(s_dve, 1)
                else:
                    _, mt, j = ev
                    if j == 0 and mt >= N_OUT:
                        # out staging slot recycled: wait for its store
                        dve.wait_ge(s_st[mt % N_OUT], 16 * (mt // N_OUT))
                    w = pe_tick[("mm", mt, KT - 1, j)]
                    ins = dve.tensor_copy(
                        outb.ap()[:, mt % N_OUT, N_OFFS[j] : N_OFFS[j] + N_SIZES[j]],
                        mm_ps[j].ap(),
                    )
                    if w > last_wait:
                        ins.wait_op(s_pe, w, "sem-ge")
                        last_wait = w
                    ins.then_inc(s_dve, 1)

    nc.finalize()
    return nc


_NC_CACHE: bass.Bass | None = None


def _get_nc() -> bass.Bass:
    global _NC_CACHE
    if _NC_CACHE is None:
        _NC_CACHE = build_bass()
    return _NC_CACHE


def run(in_maps, **spmd_kwargs):
    """Run the SPMD kernel; returns BassKernelResults."""
    return run_bass_kernel_spmd(_get_nc(), in_maps, list(range(B)), **spmd_kwargs)


def kernel(feature_A: np.ndarray, feature_B: np.ndarray) -> np.ndarray:
    feature_A = np.ascontiguousarray(np.asarray(feature_A, dtype=np.float32))
    feature_B = np.ascontiguousarray(np.asarray(feature_B, dtype=np.float32))
    assert feature_A.shape == (B, H, W, C), feature_A.shape
    assert feature_B.shape == (B, H, W, C), feature_B.shape

    in_maps = [
        {
            "a": feature_A[i].reshape(HW, C),
            "b": feature_B[i].reshape(HW, C),
        }
        for i in range(B)
    ]
    res = run(in_maps)
    out = np.stack([res.results[i]["c"].reshape(H, W, H, W) for i in range(B)])
    return out


# revision 43
# speedup vs baseline: 1.6637x; 1.6637x over previous
"""Correlation kernel for Trainium2 (raw bass, manual semaphores).

corr[b,i,j,k,l] = sum_c A[b,i,j,c] * B[b,k,l,c]

Batched GEMM per batch element: C_b = A_b @ B_b^T with A_b, B_b of
shape (HW, C) = (2304, 256), C_b of (2304, 2304). Batch (8) is
data-parallel across the 8 NeuronCores, one batch element per core.

Engine pipeline per core (all manual semaphores; the target walrus
supports only ONE inline semaphore wait per engine instruction, so any
instruction needing two conditions gets a standalone EventSemaphore):

  SP(sync)   : B loads bank-chunked (banks 0-2), stores for even
               M-tiles.
  ACT(scalar): A loads (pixel tiles 0-1 first, rest later), B bank 3-4
               loads, PSUM->SBUF copies for output banks 3-4, stores
               for odd M-tiles.
  Pool       : identity matrix build.
  PE         : 128x128 transposes of A/B into PSUM (f32r identity
               matmul, 1.5 cyc/row) + the GEMM (f32r, 2 cyc/row):
               per M-tile 5 output banks (4x512+1x256) x 2 K-subtiles.
  DVE        : transpose drains PSUM->SBUF (f32r operands) + output
               copies banks 0-2.

Dataflow: A(0),A(1) transposed first, then per B-bank [load chunk ->
transpose -> mm(mt=0)], then the M-loop (mt=1..17) with A(mt+1)
transposes prefetched one tile ahead. PSUM mm banks are recycled per
M-tile, guarded by the output copies; SBUF output staging is a 6-deep
ring guarded by per-slot store semaphores.
"""

from contextlib import ExitStack

import numpy as np

import concourse.bass as bass
import concourse.mybir as mybir
from concourse.bass import ts
from concourse.bass_utils import run_bass_kernel_spmd

# Problem shape (hardcoded; the harness always calls with these shapes).
B, H, W, C = 8, 48, 48, 256
HW = H * W  # 2304
P = 128
KT = C // P  # 2 k-subtiles
M_TILES = HW // P  # 18
N_SIZES = [512, 512, 512, 512, 256]
N_OFFS = [0, 512, 1024, 1536, 2048]
N_BANKS = len(N_SIZES)
BANK_PTS = [
    list(range(N_OFFS[j] // P, (N_OFFS[j] + N_SIZES[j]) // P)) for j in range(N_BANKS)
]
N_TP = 3  # transpose psum buffers (banks); 5 + 3 = 8 PSUM banks
N_OUT = 6  # SBUF output staging ring depth
DVE_OC_BANKS = (0, 1, 2)  # output banks copied by DVE; rest by ACT

F32 = mybir.dt.float32
F32R = mybir.dt.float32r

USE_F32R = True
MM_DT = F32R if USE_F32R else F32


def build_bass() -> bass.Bass:
    nc = bass.Bass(trn_type="TRN2")

    a_d = nc.dram_tensor("a", [HW, C], MM_DT, kind="ExternalInput")
    b_d = nc.dram_tensor("b", [HW, C], MM_DT, kind="ExternalInput")
    c_d = nc.dram_tensor("c", [HW, HW], F32, kind="ExternalOutput")

    # gpsimd memset/affine_select only handle fp32; a DVE cast then produces
    # the f32r-rounded copy the f32r transpose-matmuls are allowed to consume.
    ident = nc.alloc_sbuf_tensor("ident", [P, P], F32)
    ident_r = nc.alloc_sbuf_tensor("ident_r", [P, P], MM_DT)
    anat = nc.alloc_sbuf_tensor("anat", [P, M_TILES, C], MM_DT)
    bnat = nc.alloc_sbuf_tensor("bnat", [P, M_TILES, C], MM_DT)
    at_all = nc.alloc_sbuf_tensor("at_all", [P, KT, HW], MM_DT)
    bt_all = nc.alloc_sbuf_tensor("bt_all", [P, KT, HW], MM_DT)
    outb = nc.alloc_sbuf_tensor("outb", [P, N_OUT, HW], F32)

    mm_ps = [
        nc.alloc_psum_tensor(f"mmps{j}", [P, N_SIZES[j]], F32) for j in range(N_BANKS)
    ]
    tp_ps = [nc.alloc_psum_tensor(f"tpps{i}", [P, P], MM_DT) for i in range(N_TP)]

    # ---- Static schedule --------------------------------------------------
    # Master PE event list. ("tp", src, pt, k) / ("mm", mt, k, j).
    pe_events = []
    for k in range(KT):
        pe_events.append(("tp", "a", 0, k))
        pe_events.append(("tp", "a", 1, k))
    for j in range(N_BANKS):
        for pt in BANK_PTS[j]:
            for k in range(KT):
                pe_events.append(("tp", "b", pt, k))
        for k in range(KT):
            pe_events.append(("mm", 0, k, j))
    for mt in range(1, M_TILES):
        if mt + 1 < M_TILES:
            for k in range(KT):
                pe_events.append(("tp", "a", mt + 1, k))
        for j in range(N_BANKS):
            for k in range(KT):
                pe_events.append(("mm", mt, k, j))

    # s_pe ticks: transposes and k=1 matmuls increment.
    pe_tick = {}
    t = 0
    for ev in pe_events:
        if ev[0] == "tp" or ev[2] == KT - 1:
            t += 1
            pe_tick[ev] = t

    tpose_list = [ev[1:] for ev in pe_events if ev[0] == "tp"]
    tpose_order = {key: i for i, key in enumerate(tpose_list)}

    # Chase streams. DVE: all casts + out-copies banks 0-2. ACT: banks 3-4.
    dve_events, act_events = [], []
    for ev in pe_events:
        if ev[0] == "tp":
            dve_events.append(("cast", ev[1], ev[2], ev[3]))
        elif ev[2] == KT - 1:
            (dve_events if ev[3] in DVE_OC_BANKS else act_events).append(
                ("oc", ev[1], ev[3])
            )
    dve_tick = {ev: i + 1 for i, ev in enumerate(dve_events)}
    act_tick = {ev: i + 1 for i, ev in enumerate(act_events)}

    def cast_tick(src, pt, k):
        return dve_tick[("cast", src, pt, k)]

    def oc_dve_tick(mt, j):
        return dve_tick[("oc", mt, j)]

    def oc_act_tick(mt, j):
        return act_tick[("oc", mt, j)]

    def bank_ready_tick(j):
        return max(
            cast_tick("b", pt, k) for pt in BANK_PTS[j] for k in range(KT)
        )

    # Load plan. SP ring: B bank chunks 0..2. ACT ring: A[0:2], B banks 3-4,
    # A[2:18]. Each DMA increments the ring's load sem by 16.
    sp_loads = [("b", 0, 4), ("b", 4, 8), ("b", 8, 12)]
    act_loads = [("a", 0, 2), ("b", 12, 16), ("b", 16, 18), ("a", 2, 18)]

    all_loads = [("sp", s, lo, hi) for s, lo, hi in sp_loads] + [
        ("act", s, lo, hi) for s, lo, hi in act_loads
    ]

    def load_gate(src, pt):
        """Index into all_loads of the DMA covering pixel tile pt of src."""
        for i, (_, s, lo, hi) in enumerate(all_loads):
            if s == src and lo <= pt < hi:
                return i
        raise ValueError((src, pt))

    with (
        nc.Block() as block,
        nc.semaphore("s_pool") as s_pool,
        nc.semaphore("s_pe") as s_pe,
        nc.semaphore("s_dve") as s_dve,
        nc.semaphore("s_act") as s_act,
        ExitStack() as stack,
    ):
        s_st = [stack.enter_context(nc.semaphore(f"s_st{i}")) for i in range(N_OUT)]
        s_ld = [
            stack.enter_context(nc.semaphore(f"s_ld{i}"))
            for i in range(len(all_loads))
        ]

        def nat(src):
            return anat if src == "a" else bnat

        def nat_dram(src):
            return a_d if src == "a" else b_d

        def store_mts(parity):
            return [mt for mt in range(M_TILES) if mt % 2 == parity]

        def emit_store(eng, mt):
            eng.wait_ge(s_dve, oc_dve_tick(mt, max(DVE_OC_BANKS)))
            eng.wait_ge(s_act, oc_act_tick(mt, N_BANKS - 1))
            eng.dma_start(
                out=c_d.ap()[ts(mt, P), :], in_=outb.ap()[:, mt % N_OUT, :]
            ).then_inc(s_st[mt % N_OUT], 16)

        def emit_load(eng, li):
            _, src, lo, hi = all_loads[li]
            eng.dma_start(
                out=nat(src).ap()[:, lo:hi, :],
                in_=nat_dram(src)
                .ap()[lo * P : hi * P, :]
                .rearrange("(po pi) c -> pi po c", pi=P),
            ).then_inc(s_ld[li], 16)

        @block.sync
        def _(sync: bass.BassEngine):
            for li, (ring, _, _, _) in enumerate(all_loads):
                if ring == "sp":
                    emit_load(sync, li)
            for mt in store_mts(0):
                emit_store(sync, mt)

        @block.gpsimd
        def _(gpsimd: bass.BassEngine):
            gpsimd.memset(ident.ap(), 0.0).then_inc(s_pool, 1)
            gpsimd.affine_select(
                out=ident.ap(),
                in_=ident.ap(),
                compare_op=mybir.AluOpType.not_equal,
                fill=1.0,
                base=0,
                # out[x, y] = (x - y) != 0 ? 0.0 : 1.0
                pattern=[[-1, P]],
                channel_multiplier=1,
            ).wait_op(s_pool, 1, "sem-ge").then_inc(s_pool, 1)

        @block.tensor
        def _(pe: bass.BassEngine):
            pe.wait_ge(s_pool, 3)
            seen_gate = set()
            last = {}  # sem name -> last waited value

            def inline_wait(ins, sem, name, val):
                if val > last.get(name, 0):
                    ins.wait_op(sem, val, "sem-ge")
                    last[name] = val

            for ev in pe_events:
                if ev[0] == "tp":
                    _, src, pt, k = ev
                    li = load_gate(src, pt)
                    if li not in seen_gate:
                        pe.wait_ge(s_ld[li], 16)
                        seen_gate.add(li)
                    i = tpose_order[(src, pt, k)]
                    ins = pe.transpose(
                        tp_ps[i % N_TP].ap(),
                        nat(src).ap()[:, pt, ts(k, P)],
                        ident_r.ap(),
                    )
                    if i >= N_TP:
                        inline_wait(ins, s_dve, "dve", cast_tick(*tpose_list[i - N_TP]))
                    ins.then_inc(s_pe, 1)
                else:
                    _, mt, k, j = ev
                    ins = pe.matmul(
                        mm_ps[j].ap(),
                        at_all.ap()[:, k, ts(mt, P)],
                        bt_all.ap()[:, k, N_OFFS[j] : N_OFFS[j] + N_SIZES[j]],
                        start=(k == 0),
                        stop=(k == KT - 1),
                    )
                    need_dve = max(
                        cast_tick("a", mt, 0),
                        cast_tick("a", mt, 1),
                        bank_ready_tick(j),
                        (
                            oc_dve_tick(mt - 1, j)
                            if (mt > 0 and j in DVE_OC_BANKS)
                            else 0
                        ),
                    )
                    inline_wait(ins, s_dve, "dve", need_dve)
                    if mt > 0 and j not in DVE_OC_BANKS:
                        inline_wait(ins, s_act, "act", oc_act_tick(mt - 1, j))
                    if k == KT - 1:
                        ins.then_inc(s_pe, 1)

        @block.vector
        def _(dve: bass.BassEngine):
            dve.tensor_copy(ident_r.ap(), ident.ap()).wait_op(
                s_pool, 2, "sem-ge"
            ).then_inc(s_pool, 1)
            last = [0]

            def inline_pe_wait(ins, val):
                if val > last[0]:
                    ins.wait_op(s_pe, val, "sem-ge")
                    last[0] = val

            for ev in dve_events:
                if ev[0] == "cast":
                    _, src, pt, k = ev
                    i = tpose_order[(src, pt, k)]
                    dst = (at_all if src == "a" else bt_all).ap()[:, k, ts(pt, P)]
                    ins = dve.tensor_copy(dst, tp_ps[i % N_TP].ap())
                    inline_pe_wait(ins, pe_tick[("tp", src, pt, k)])
                    ins.then_inc(s_dve, 1)
                else:
                    _, mt, j = ev
                    if j == DVE_OC_BANKS[0] and mt >= N_OUT:
                        dve.wait_ge(s_st[mt % N_OUT], 16 * (mt // N_OUT))
                    ins = dve.tensor_copy(
                        outb.ap()[:, mt % N_OUT, N_OFFS[j] : N_OFFS[j] + N_SIZES[j]],
                        mm_ps[j].ap(),
                    )
                    inline_pe_wait(ins, pe_tick[("mm", mt, KT - 1, j)])
                    ins.then_inc(s_dve, 1)

        @block.scalar
        def _(act: bass.BassEngine):
            for li, (ring, _, _, _) in enumerate(all_loads):
                if ring == "act":
                    emit_load(act, li)
            last = [0]
            odd = store_mts(1)
            oi = 0
            for ev in act_events:
                _, mt, j = ev
                if j == min(b for b in range(N_BANKS) if b not in DVE_OC_BANKS):
                    if mt >= N_OUT:
                        act.wait_ge(s_st[mt % N_OUT], 16 * (mt // N_OUT))
                ins = act.copy(
                    outb.ap()[:, mt % N_OUT, N_OFFS[j] : N_OFFS[j] + N_SIZES[j]],
                    mm_ps[j].ap(),
                )
                v = pe_tick[("mm", mt, KT - 1, j)]
                if v > last[0]:
                    ins.wait_op(s_pe, v, "sem-ge")
                    last[0] = v
                ins.then_inc(s_act, 1)
                if j == N_BANKS - 1 and oi < len(odd) and odd[oi] == mt:
                    emit_store(act, mt)
                    oi += 1

    nc.finalize()
    return nc


_NC_CACHE: bass.Bass | None = None


def _get_nc() -> bass.Bass:
    global _NC_CACHE
    if _NC_CACHE is None:
        _NC_CACHE = build_bass()
    return _NC_CACHE


def run(in_maps, **spmd_kwargs):
    """Run the SPMD kernel; returns BassKernelResults."""
    return run_bass_kernel_spmd(_get_nc(), in_maps, list(range(B)), **spmd_kwargs)


def kernel(feature_A: np.ndarray, feature_B: np.ndarray) -> np.ndarray:
    feature_A = np.ascontiguousarray(np.asarray(feature_A, dtype=np.float32))
    feature_B = np.ascontiguousarray(np.asarray(feature_B, dtype=np.float32))
    assert feature_A.shape == (B, H, W, C), feature_A.shape
    assert feature_B.shape == (B, H, W, C), feature_B.shape

    in_maps = [
        {
            "a": feature_A[i].reshape(HW, C),
            "b": feature_B[i].reshape(HW, C),
        }
        for i in range(B)
    ]
    res = run(in_maps)
    out = np.stack([res.results[i]["c"].reshape(H, W, H, W) for i in range(B)])
    return out


# revision 51
# speedup vs baseline: 1.6899x; 1.0158x over previous
"""Correlation kernel for Trainium2 (raw bass, manual semaphores).

corr[b,i,j,k,l] = sum_c A[b,i,j,c] * B[b,k,l,c]

Batched GEMM per batch element: C_b = A_b @ B_b^T with A_b, B_b of
shape (HW, C) = (2304, 256), C_b of (2304, 2304). Batch (8) is
data-parallel across the 8 NeuronCores, one batch element per core.

Engine pipeline per core (all manual semaphores; the target walrus
supports only ONE inline semaphore wait per engine instruction, so any
instruction needing two conditions gets a standalone EventSemaphore):

  SP(sync)   : B loads bank-chunked (banks 0-2), stores for even
               M-tiles.
  ACT(scalar): A loads (pixel tiles 0-1 first, rest later), B bank 3-4
               loads, PSUM->SBUF copies for output banks 3-4, stores
               for odd M-tiles.
  Pool       : identity matrix build.
  PE         : 128x128 transposes of A/B into PSUM (f32r identity
               matmul, 1.5 cyc/row) + the GEMM (f32r, 2 cyc/row):
               per M-tile 5 output banks (4x512+1x256) x 2 K-subtiles.
  DVE        : transpose drains PSUM->SBUF (f32r operands) + output
               copies banks 0-2.

Dataflow: A(0),A(1) transposed first, then per B-bank [load chunk ->
transpose -> mm(mt=0)], then the M-loop (mt=1..17) with A(mt+1)
transposes prefetched one tile ahead. PSUM mm banks are recycled per
M-tile, guarded by the output copies; SBUF output staging is a 6-deep
ring guarded by per-slot store semaphores.
"""

from contextlib import ExitStack

import numpy as np

import concourse.bass as bass
import concourse.mybir as mybir
from concourse.bass import ts
from concourse.bass_utils import run_bass_kernel_spmd

# Problem shape (hardcoded; the harness always calls with these shapes).
B, H, W, C = 8, 48, 48, 256
HW = H * W  # 2304
P = 128
KT = C // P  # 2 k-subtiles
M_TILES = HW // P  # 18
N_SIZES = [512, 512, 512, 512, 256]
N_OFFS = [0, 512, 1024, 1536, 2048]
N_BANKS = len(N_SIZES)
BANK_PTS = [
    list(range(N_OFFS[j] // P, (N_OFFS[j] + N_SIZES[j]) // P)) for j in range(N_BANKS)
]
N_TP = 3  # transpose psum buffers (banks); 5 + 3 = 8 PSUM banks
N_OUT = 6  # SBUF output staging ring depth
DVE_OC_BANKS = (0, 1, 2)  # output banks copied by DVE; rest by ACT

F32 = mybir.dt.float32
F32R = mybir.dt.float32r

USE_F32R = True
MM_DT = F32R if USE_F32R else F32


def build_bass() -> bass.Bass:
    nc = bass.Bass(trn_type="TRN2")

    a_d = nc.dram_tensor("a", [HW, C], MM_DT, kind="ExternalInput")
    b_d = nc.dram_tensor("b", [HW, C], MM_DT, kind="ExternalInput")
    c_d = nc.dram_tensor("c", [HW, HW], F32, kind="ExternalOutput")

    # gpsimd memset/affine_select only handle fp32; a DVE cast then produces
    # the f32r-rounded copy the f32r transpose-matmuls are allowed to consume.
    ident = nc.alloc_sbuf_tensor("ident", [P, P], F32)
    ident_r = nc.alloc_sbuf_tensor("ident_r", [P, P], MM_DT)
    anat = nc.alloc_sbuf_tensor("anat", [P, M_TILES, C], MM_DT)
    bnat = nc.alloc_sbuf_tensor("bnat", [P, M_TILES, C], MM_DT)
    at_all = nc.alloc_sbuf_tensor("at_all", [P, KT, HW], MM_DT)
    bt_all = nc.alloc_sbuf_tensor("bt_all", [P, KT, HW], MM_DT)
    outb = nc.alloc_sbuf_tensor("outb", [P, N_OUT, HW], F32)

    mm_ps = [
        nc.alloc_psum_tensor(f"mmps{j}", [P, N_SIZES[j]], F32) for j in range(N_BANKS)
    ]
    tp_ps = [nc.alloc_psum_tensor(f"tpps{i}", [P, P], MM_DT) for i in range(N_TP)]

    # ---- Static schedule --------------------------------------------------
    # Master PE event list. ("tp", src, pt, k) / ("mm", mt, k, j).
    pe_events = []
    for k in range(KT):
        pe_events.append(("tp", "a", 0, k))
        pe_events.append(("tp", "a", 1, k))
    # Bank passes: transpose bank j+1 BEFORE mm(0, bank j) so the mm overlaps
    # the next bank's transpose drains.
    for pt in BANK_PTS[0]:
        for k in range(KT):
            pe_events.append(("tp", "b", pt, k))
    for j in range(N_BANKS):
        if j + 1 < N_BANKS:
            for pt in BANK_PTS[j + 1]:
                for k in range(KT):
                    pe_events.append(("tp", "b", pt, k))
        for k in range(KT):
            pe_events.append(("mm", 0, k, j))
    for mt in range(1, M_TILES):
        if mt + 1 < M_TILES:
            for k in range(KT):
                pe_events.append(("tp", "a", mt + 1, k))
        for j in range(N_BANKS):
            for k in range(KT):
                pe_events.append(("mm", mt, k, j))

    # s_pe ticks: transposes and k=1 matmuls increment.
    pe_tick = {}
    t = 0
    for ev in pe_events:
        if ev[0] == "tp" or ev[2] == KT - 1:
            t += 1
            pe_tick[ev] = t

    tpose_list = [ev[1:] for ev in pe_events if ev[0] == "tp"]
    tpose_order = {key: i for i, key in enumerate(tpose_list)}

    # Chase streams. Casts of k=0 transposes drain on DVE, k=1 on ACT (so a
    # k-subtile matmul needs only its own stream's semaphore). Out-copies:
    # banks 0-2 on DVE, 3-4 on ACT.
    def chase_engine(ev):
        if ev[0] == "cast":
            return "dve" if ev[3] == 0 else "act"
        return "dve" if ev[2] in DVE_OC_BANKS else "act"

    dve_events, act_events = [], []
    for ev in pe_events:
        if ev[0] == "tp":
            ch = ("cast", ev[1], ev[2], ev[3])
        elif ev[2] == KT - 1:
            ch = ("oc", ev[1], ev[3])
        else:
            continue
        (dve_events if chase_engine(ch) == "dve" else act_events).append(ch)
    tick = {}
    for name, evs in (("dve", dve_events), ("act", act_events)):
        for i, ev in enumerate(evs):
            tick[ev] = (name, i + 1)

    def cast_need(src, pt, k):
        return tick[("cast", src, pt, k)]

    def oc_need(mt, j):
        return tick[("oc", mt, j)]

    def bank_ready_need(j, k):
        sem, val = None, 0
        for pt in BANK_PTS[j]:
            s, v = tick[("cast", "b", pt, k)]
            sem, val = s, max(val, v)
        return sem, val

    # Load plan. SP ring: B bank chunks 0..2. ACT ring: A[0:2], B banks 3-4,
    # A[2:18]. Each DMA increments the ring's load sem by 16.
    sp_loads = [("b", 0, 4), ("b", 4, 8), ("b", 8, 12)]
    act_loads = [("a", 0, 2), ("b", 12, 16), ("b", 16, 18), ("a", 2, 18)]

    all_loads = [("sp", s, lo, hi) for s, lo, hi in sp_loads] + [
        ("act", s, lo, hi) for s, lo, hi in act_loads
    ]

    def load_gate(src, pt):
        """Index into all_loads of the DMA covering pixel tile pt of src."""
        for i, (_, s, lo, hi) in enumerate(all_loads):
            if s == src and lo <= pt < hi:
                return i
        raise ValueError((src, pt))

    with (
        nc.Block() as block,
        nc.semaphore("s_pool") as s_pool,
        nc.semaphore("s_pe") as s_pe,
        nc.semaphore("s_dve") as s_dve,
        nc.semaphore("s_act") as s_act,
        ExitStack() as stack,
    ):
        s_st = [stack.enter_context(nc.semaphore(f"s_st{i}")) for i in range(N_OUT)]
        s_ld = [
            stack.enter_context(nc.semaphore(f"s_ld{i}"))
            for i in range(len(all_loads))
        ]
        sems = {"dve": s_dve, "act": s_act}

        def nat(src):
            return anat if src == "a" else bnat

        def nat_dram(src):
            return a_d if src == "a" else b_d

        def store_mts(parity):
            return [mt for mt in range(M_TILES) if mt % 2 == parity]

        def emit_store(eng, mt):
            for j in (max(DVE_OC_BANKS), N_BANKS - 1):
                sname, val = oc_need(mt, j)
                eng.wait_ge(sems[sname], val)
            eng.dma_start(
                out=c_d.ap()[ts(mt, P), :], in_=outb.ap()[:, mt % N_OUT, :]
            ).then_inc(s_st[mt % N_OUT], 16)

        def emit_load(eng, li):
            _, src, lo, hi = all_loads[li]
            eng.dma_start(
                out=nat(src).ap()[:, lo:hi, :],
                in_=nat_dram(src)
                .ap()[lo * P : hi * P, :]
                .rearrange("(po pi) c -> pi po c", pi=P),
            ).then_inc(s_ld[li], 16)

        @block.sync
        def _(sync: bass.BassEngine):
            for li, (ring, _, _, _) in enumerate(all_loads):
                if ring == "sp":
                    emit_load(sync, li)
            for mt in store_mts(0):
                emit_store(sync, mt)

        @block.gpsimd
        def _(gpsimd: bass.BassEngine):
            gpsimd.memset(ident.ap(), 0.0).then_inc(s_pool, 1)
            gpsimd.affine_select(
                out=ident.ap(),
                in_=ident.ap(),
                compare_op=mybir.AluOpType.not_equal,
                fill=1.0,
                base=0,
                # out[x, y] = (x - y) != 0 ? 0.0 : 1.0
                pattern=[[-1, P]],
                channel_multiplier=1,
            ).wait_op(s_pool, 1, "sem-ge").then_inc(s_pool, 1)

        @block.tensor
        def _(pe: bass.BassEngine):
            pe.wait_ge(s_pool, 3)
            seen_gate = set()
            last = {"dve": 0, "act": 0}

            def reduce_needs(needs):
                """Collapse (sem,val) needs to the unmet ones, max per sem."""
                m = {}
                for sname, val in needs:
                    if val > last[sname]:
                        m[sname] = max(m.get(sname, 0), val)
                return m

            def apply_needs(ins, m, pre_emitted):
                # pre_emitted were issued as standalone waits before `ins`;
                # the remaining (at most one) rides inline.
                for sname, val in m.items():
                    if sname in pre_emitted:
                        continue
                    ins.wait_op(sems[sname], val, "sem-ge")
                for sname, val in m.items():
                    last[sname] = max(last[sname], val)

            for ev in pe_events:
                if ev[0] == "tp":
                    _, src, pt, k = ev
                    li = load_gate(src, pt)
                    if li not in seen_gate:
                        pe.wait_ge(s_ld[li], 16)
                        seen_gate.add(li)
                    i = tpose_order[(src, pt, k)]
                    needs = []
                    if i >= N_TP:
                        needs.append(cast_need(*tpose_list[i - N_TP]))
                    m = reduce_needs(needs)
                    pre = set()
                    while len(m) - len(pre) > 1:
                        sname = next(s for s in m if s not in pre)
                        pe.wait_ge(sems[sname], m[sname])
                        pre.add(sname)
                    ins = pe.transpose(
                        tp_ps[i % N_TP].ap(),
                        nat(src).ap()[:, pt, ts(k, P)],
                        ident_r.ap(),
                    )
                    apply_needs(ins, m, pre)
                    ins.then_inc(s_pe, 1)
                else:
                    _, mt, k, j = ev
                    needs = [
                        cast_need("a", mt, k),
                        bank_ready_need(j, k),
                    ]
                    if mt > 0:
                        needs.append(oc_need(mt - 1, j))
                    m = reduce_needs(needs)
                    pre = set()
                    while len(m) - len(pre) > 1:
                        sname = next(s for s in m if s not in pre)
                        pe.wait_ge(sems[sname], m[sname])
                        pre.add(sname)
                    ins = pe.matmul(
                        mm_ps[j].ap(),
                        at_all.ap()[:, k, ts(mt, P)],
                        bt_all.ap()[:, k, N_OFFS[j] : N_OFFS[j] + N_SIZES[j]],
                        start=(k == 0),
                        stop=(k == KT - 1),
                    )
                    apply_needs(ins, m, pre)
                    if k == KT - 1:
                        ins.then_inc(s_pe, 1)

        ACT_OC_BANKS = [b for b in range(N_BANKS) if b not in DVE_OC_BANKS]

        def chase_block(eng, events, own_sem, copy_fn, do_stores):
            last = [0]

            def inline_pe_wait(ins, val):
                if val > last[0]:
                    ins.wait_op(s_pe, val, "sem-ge")
                    last[0] = val

            first_oc = (DVE_OC_BANKS if not do_stores else ACT_OC_BANKS)[0]
            for ev in events:
                if ev[0] == "cast":
                    _, src, pt, k = ev
                    i = tpose_order[(src, pt, k)]
                    dst = (at_all if src == "a" else bt_all).ap()[:, k, ts(pt, P)]
                    ins = copy_fn(dst, tp_ps[i % N_TP].ap())
                    inline_pe_wait(ins, pe_tick[("tp", src, pt, k)])
                    ins.then_inc(own_sem, 1)
                else:
                    _, mt, j = ev
                    if j == first_oc and mt >= N_OUT:
                        eng.wait_ge(s_st[mt % N_OUT], 16 * (mt // N_OUT))
                    ins = copy_fn(
                        outb.ap()[:, mt % N_OUT, N_OFFS[j] : N_OFFS[j] + N_SIZES[j]],
                        mm_ps[j].ap(),
                    )
                    inline_pe_wait(ins, pe_tick[("mm", mt, KT - 1, j)])
                    ins.then_inc(own_sem, 1)
                    if do_stores and j == N_BANKS - 1 and mt % 2 == 1:
                        emit_store(eng, mt)

        @block.vector
        def _(dve: bass.BassEngine):
            dve.tensor_copy(ident_r.ap(), ident.ap()).wait_op(
                s_pool, 2, "sem-ge"
            ).then_inc(s_pool, 1)
            chase_block(dve, dve_events, s_dve, dve.tensor_copy, do_stores=False)

        @block.scalar
        def _(act: bass.BassEngine):
            for li, (ring, _, _, _) in enumerate(all_loads):
                if ring == "act":
                    emit_load(act, li)
            chase_block(act, act_events, s_act, act.copy, do_stores=True)

    nc.finalize()
    return nc


_NC_CACHE: bass.Bass | None = None


def _get_nc() -> bass.Bass:
    global _NC_CACHE
    if _NC_CACHE is None:
        _NC_CACHE = build_bass()
    return _NC_CACHE


def run(in_maps, **spmd_kwargs):
    """Run the SPMD kernel; returns BassKernelResults."""
    return run_bass_kernel_spmd(_get_nc(), in_maps, list(range(B)), **spmd_kwargs)


def kernel(feature_A: np.ndarray, feature_B: np.ndarray) -> np.ndarray:
    feature_A = np.ascontiguousarray(np.asarray(feature_A, dtype=np.float32))
    feature_B = np.ascontiguousarray(np.asarray(feature_B, dtype=np.float32))
    assert feature_A.shape == (B, H, W, C), feature_A.shape
    assert feature_B.shape == (B, H, W, C), feature_B.shape

    in_maps = [
        {
            "a": feature_A[i].reshape(HW, C),
            "b": feature_B[i].reshape(HW, C),
        }
        for i in range(B)
    ]
    res = run(in_maps)
    out = np.stack([res.results[i]["c"].reshape(H, W, H, W) for i in range(B)])
    return out
